# revision 13
# baseline (speedup 1.0000x reference)
"""GCN autoencoder kernel, P=16 packed-gather version, 8 TRN2 cores.

Each DMA-gather descriptor fetches a 1KB row = 16 nodes x 32 feats (bf16),
cutting gather descriptors/ring packets ~7x vs per-edge rows. Edge weights
are separable (w = a[dst]*b[src]): b is folded into the stored tables, a into
the post-aggregation activations, so the per-(chunk,phase) selection matrices
are exact small integers stored as fp8.

Aggregation per dst m-tile: psum[128 dst, 32] accumulates chunk x phase
matmuls: lhsT = smat[:, chunk, phase, :] (fp8 counts), rhs = gathered
gb[:, chunk, 32p:32p+32] (bf16 values).

Table layout ("wrapped"): node n -> core c=n//1250, local=n%1250, mt=local//128,
p=local%128; 64B unit u = c*1280 + p*10 + mt; row u//16, phase u%16. This
makes the per-core staged tile [128, MT*32] a single contiguous DMA store.

Decode: out = sigmoid(z_own @ z_all^T) in bf16, stores alternating SP/Act
HWDGE queues; host upcasts to f32.
"""

from contextlib import ExitStack
from dataclasses import dataclass, field

import ml_dtypes
import numpy as np

import concourse.bass as bass  # noqa: F401
import concourse.mybir as mybir
import concourse.tile as tile
from concourse import bacc
from concourse.bass_utils import run_bass_kernel_spmd

dt = mybir.dt

N = 10000
R = 1250
MT = 10
NCORES = 8
NFEAT = 512
HID = 32
CODE = 16
P = 16
ELEM = P * HID  # 512 bf16 = 1KB per gather row
NROWS = NCORES * 1280 // P  # 640


@dataclass
class Cfg:
    n_queues: int = 4
    CMU: tuple = ()  # chunks per m-tile (uniform across cores)

    @property
    def cbase(self):
        b, acc = [], 0
        for c in self.CMU:
            b.append(acc)
            acc += c
        return b

    @property
    def sumcm(self):
        return sum(self.CMU)

    @property
    def kch(self):
        return NFEAT // 128


def rows_of(m):
    return min(128, R - m * 128)


def prep(cfg: Cfg, src, dst, ew):
    src = np.asarray(src).astype(np.int64)
    dst = np.asarray(dst).astype(np.int64)
    ew64 = np.asarray(ew).astype(np.float64)

    deg_row = np.bincount(dst, minlength=N).astype(np.float64)
    deg_col = np.bincount(src, minlength=N).astype(np.float64)
    a = 1.0 / np.sqrt(deg_row)
    b = 1.0 / np.sqrt(deg_col)
    w_check = (a[dst] * b[src]).astype(np.float32)
    rel = np.abs(w_check - ew64.astype(np.float32)) / np.maximum(np.abs(ew64), 1e-9)
    assert rel.max() < 1e-4, f"edge_weight not separable: rel={rel.max()}"
    a = a.astype(np.float32)
    b = b.astype(np.float32)

    u = (src // R) * 1280 + (src % R % 128) * 10 + (src % R // 128)
    row = u // P
    ph = u % P

    CM = np.zeros((NCORES, MT), np.int64)
    per_core = []
    for c in range(NCORES):
        m = (dst >= c * R) & (dst < (c + 1) * R)
        per_core.append((row[m], ph[m], dst[m] - c * R))
        for mt in range(MT):
            sel = (per_core[c][2] // 128) == mt
            CM[c, mt] = -(-len(np.unique(per_core[c][0][sel])) // 128)
    cfg.CMU = tuple(int(v) for v in CM.max(axis=0))
    CBASE = cfg.cbase
    SLOTS = cfg.sumcm * 128

    maps = []
    for c in range(NCORES):
        e_row, e_ph, e_dl = per_core[c]
        gidx_lin = np.zeros(SLOTS, np.int64)
        smat = np.zeros((128, cfg.sumcm, P, 128), np.float32)
        for mt in range(MT):
            sel = (e_dl // 128) == mt
            rows_mt = e_row[sel]
            uniq, inv = np.unique(rows_mt, return_inverse=True)
            base = CBASE[mt] * 128
            gidx_lin[base : base + len(uniq)] = uniq
            pos = inv % 128
            ch = CBASE[mt] + inv // 128
            d128 = e_dl[sel] - mt * 128
            np.add.at(smat, (pos, ch, e_ph[sel], d128), 1.0)
        gidx = np.tile(gidx_lin.reshape(-1, 16).T.astype(np.int16), (8, 1))
        bcol = np.zeros((128, MT), np.float32)
        acol = np.zeros((128, MT), np.float32)
        for mt in range(MT):
            rm = rows_of(mt)
            bcol[:rm, mt] = b[c * R + mt * 128 : c * R + mt * 128 + rm]
            acol[:rm, mt] = a[c * R + mt * 128 : c * R + mt * 128 + rm]
        maps.append(
            dict(
                gidx=np.ascontiguousarray(gidx),
                smat=np.ascontiguousarray(smat.astype(ml_dtypes.float8_e4m3fn)),
                bcol=bcol,
                acol=acol,
                abcol=np.ascontiguousarray(acol * bcol),
            )
        )
    return maps


def build_nc(cfg: Cfg):
    nc = bacc.Bacc(
        "TRN2",
        target_bir_lowering=False,
        debug=False,
        enable_asserts=False,
        num_devices=NCORES,
        num_swdge_queues=cfg.n_queues,
    )
    f32 = dt.float32
    bf16 = dt.bfloat16
    fp8 = dt.float8e4
    KCH = cfg.kch
    CMU, CBASE, SUMCM = cfg.CMU, cfg.cbase, cfg.sumcm
    CMX = max(CMU)
    AF = mybir.ActivationFunctionType

    xs = nc.dram_tensor("xs", [R, NFEAT], f32, kind="ExternalInput").ap()
    w1 = nc.dram_tensor("w1", [NFEAT, HID], f32, kind="ExternalInput").ap()
    w2 = nc.dram_tensor("w2", [HID, CODE], f32, kind="ExternalInput").ap()
    ident_d = nc.dram_tensor("ident", [128, 128], f32, kind="ExternalInput").ap()
    gidx_d = nc.dram_tensor(
        "gidx", [128, SUMCM * 128 // 16], dt.int16, kind="ExternalInput"
    ).ap()
    smat_d = nc.dram_tensor(
        "smat", [128, SUMCM, P, 128], fp8, kind="ExternalInput"
    ).ap()
    bcol_d = nc.dram_tensor("bcol", [128, MT], f32, kind="ExternalInput").ap()
    acol_d = nc.dram_tensor("acol", [128, MT], f32, kind="ExternalInput").ap()
    abcol_d = nc.dram_tensor("abcol", [128, MT], f32, kind="ExternalInput").ap()
    out_d = nc.dram_tensor("out", [R, N], bf16, kind="ExternalOutput").ap()

    y1_own = nc.dram_tensor("y1_own", [128, MT * HID], bf16).ap()
    y1_all = nc.dram_tensor(
        "y1_all", [NCORES, 128, MT * HID], bf16, addr_space="Shared"
    ).ap()
    h_own = nc.dram_tensor("h_own", [128, MT * HID], bf16).ap()
    h_all = nc.dram_tensor(
        "h_all", [NCORES, 128, MT * HID], bf16, addr_space="Shared"
    ).ap()
    zt_own = nc.dram_tensor("zt_own", [CODE, R], bf16).ap()
    zt_all = nc.dram_tensor(
        "zt_all", [NCORES, CODE, R], bf16, addr_space="Shared"
    ).ap()

    groups_all = [list(range(NCORES))]
    y1_tab = y1_all.rearrange("c p f -> (c p f)").rearrange("(r e) -> r e", e=ELEM)
    h_tab = h_all.rearrange("c p f -> (c p f)").rearrange("(r e) -> r e", e=ELEM)

    # decode N-chunking: 512-wide chunks grouped 4 per PSUM tile
    nchunks = []
    n0 = 0
    while n0 < N:
        nn = min(512, N - n0)
        nchunks.append((n0, nn))
        n0 += nn
    bank_groups = [nchunks[i : i + 4] for i in range(0, len(nchunks), 4)]

    with tile.TileContext(nc) as tc, ExitStack() as ctx:
        cpool = ctx.enter_context(tc.tile_pool(name="consts", bufs=1))
        spool = ctx.enter_context(tc.tile_pool(name="smat", bufs=1))
        zpool = ctx.enter_context(tc.tile_pool(name="zbits", bufs=1))

        ident = cpool.tile([128, 128], f32)
        nc.sync.dma_start(ident[:], ident_d[:, :])
        w1s = cpool.tile([128, KCH, HID], f32)
        for k in range(KCH):
            nc.sync.dma_start(w1s[:, k, :], w1[k * 128 : (k + 1) * 128, :])
        w2s = cpool.tile([HID, CODE], f32)
        nc.sync.dma_start(w2s[:], w2[:, :])
        bcol = cpool.tile([128, MT], f32)
        nc.sync.dma_start(bcol[:], bcol_d[:, :])
        acol = cpool.tile([128, MT], f32)
        nc.sync.dma_start(acol[:], acol_d[:, :])
        abcol = cpool.tile([128, MT], f32)
        nc.sync.dma_start(abcol[:], abcol_d[:, :])
        gidx = cpool.tile([128, SUMCM * 128 // 16], dt.int16)
        nc.scalar.dma_start(gidx[:], gidx_d[:, :])
        smat = spool.tile([128, SUMCM, P, 128], fp8)
        half = SUMCM // 2
        nc.scalar.dma_start(smat[:, :half, :, :], smat_d[:, :half, :, :])
        nc.scalar.dma_start(smat[:, half:, :, :], smat_d[:, half:, :, :])

        zts = zpool.tile([CODE, R], bf16)
        zts4 = zpool.tile([128, R], bf16)
        ztall4 = zpool.tile([128, N], bf16)
        zpreT = zpool.tile([HID, MT * 128], f32)

        # ============ phase A/B: x^T, Y1 = b * (x @ W1), wrapped staging ====
        with tc.tile_pool(name="xio", bufs=2) as xio, tc.tile_pool(
            name="xt", bufs=1
        ) as xtp, tc.tile_pool(name="pst", bufs=2, space="PSUM") as pst, tc.tile_pool(
            name="psy", bufs=2, space="PSUM"
        ) as psy, tc.tile_pool(name="ystage", bufs=1) as ysp:
            xT = xtp.tile([128, KCH, MT * 128], f32)
            for m in range(MT):
                rm = rows_of(m)
                xin = xio.tile([128, NFEAT], f32)
                nc.sync.dma_start(xin[:rm, :], xs[m * 128 : m * 128 + rm, :])
                for k in range(KCH):
                    pt = pst.tile([128, 128], f32, space="PSUM")
                    nc.tensor.transpose(
                        pt[:, :rm],
                        xin[:rm, k * 128 : (k + 1) * 128],
                        ident[:rm, :rm],
                    )
                    nc.vector.tensor_copy(
                        xT[:, k, m * 128 : m * 128 + rm], pt[:, :rm]
                    )
            yst = ysp.tile([128, MT, HID], bf16)
            nc.vector.memset(yst[:, :, :], 0.0)
            for m in range(MT):
                rm = rows_of(m)
                py = psy.tile([128, HID], f32, space="PSUM")
                for k in range(KCH):
                    nc.tensor.matmul(
                        py[:rm, :],
                        lhsT=xT[:, k, m * 128 : m * 128 + rm],
                        rhs=w1s[:, k, :],
                        start=(k == 0),
                        stop=(k == KCH - 1),
                    )
                nc.scalar.activation(
                    yst[:rm, m, :], py[:rm, :], AF.Copy, scale=bcol[:rm, m : m + 1]
                )
            nc.sync.dma_start(
                y1_own.rearrange("p (m f) -> p m f", m=MT), yst[:, :, :]
            )

        nc.gpsimd.collective_compute(
            "AllGather",
            mybir.AluOpType.bypass,
            replica_groups=groups_all,
            ins=[y1_own.opt()],
            outs=[y1_all.opt()],
        )

        # ============ packed SpMM ============
        # each m-tile's gather is split across 2 SWDGE queues so a queue's
        # ring drains 2x faster and the next desc-gen doesn't stall on space
        qq_state = [0]  # queue rotation carried across layers

        def spmm(tab, emit, tag):
            with tc.tile_pool(name=f"gb_{tag}", bufs=3) as gpool, tc.tile_pool(
                name=f"ps_{tag}", bufs=3, space="PSUM"
            ) as psg:
                for mt in range(MT):
                    cmm, cb = CMU[mt], CBASE[mt]
                    gb = gpool.tile([128, CMX, ELEM], bf16, tag="gb")
                    ca = cmm // 2
                    for c0, cn in ((0, ca), (ca, cmm - ca)):
                        nidx = cn * 128
                        nc.gpsimd.dma_gather(
                            out_ap=gb[:, c0 : c0 + cn, :],
                            in_ap=tab[:, :],
                            idxs_ap=gidx[:, (cb + c0) * 8 : (cb + c0 + cn) * 8],
                            num_idxs=nidx,
                            num_idxs_reg=nidx,
                            elem_size=ELEM,
                            single_packet=False,
                            queue_num=qq_state[0] % cfg.n_queues,
                        )
                        qq_state[0] += 1
                    pm = psg.tile([128, HID], f32, space="PSUM", tag="pm")
                    k, last = 0, cmm * P - 1
                    for c in range(cmm):
                        for p in range(P):
                            nc.tensor.matmul(
                                pm[:, :],
                                lhsT=smat[:, cb + c, p, :],
                                rhs=gb[:, c, 32 * p : 32 * p + 32],
                                start=(k == 0),
                                stop=(k == last),
                            )
                            k += 1
                    emit(mt, pm)

        # ---- layer 1: h' = relu(a*b*pre), wrapped staging + AllGather ----
        with tc.tile_pool(name="hstage", bufs=1) as hsp:
            hst = hsp.tile([128, MT, HID], bf16)
            nc.vector.memset(hst[:, :, :], 0.0)

            def l1_out(mt, pm):
                rm = rows_of(mt)
                nc.scalar.activation(
                    hst[:rm, mt, :],
                    pm[:rm, :],
                    AF.Relu,
                    scale=abcol[:rm, mt : mt + 1],
                )

            spmm(y1_tab, l1_out, "l1")
            nc.sync.dma_start(
                h_own.rearrange("p (m f) -> p m f", m=MT), hst[:, :, :]
            )

        nc.gpsimd.collective_compute(
            "AllGather",
            mybir.AluOpType.bypass,
            replica_groups=groups_all,
            ins=[h_own.opt()],
            outs=[h_all.opt()],
        )

        # ---- layer 2: zpre = a*pre, transposed; z^T = W2^T @ zpre^T ----
        with tc.tile_pool(name="zstage", bufs=2) as zstage, tc.tile_pool(
            name="pstz", bufs=2, space="PSUM"
        ) as pstz:

            def l2_out(mt, pm):
                rm = rows_of(mt)
                zp = zstage.tile([128, HID], f32)
                nc.scalar.activation(
                    zp[:rm, :], pm[:rm, :], AF.Copy, scale=acol[:rm, mt : mt + 1]
                )
                ptz = pstz.tile([HID, 128], f32, space="PSUM")
                nc.tensor.transpose(ptz[:, :rm], zp[:rm, :], ident[:rm, :rm])
                nc.vector.tensor_copy(zpreT[:, mt * 128 : mt * 128 + rm], ptz[:, :rm])

            spmm(h_tab, l2_out, "l2")

            zn0 = 0
            while zn0 < R:
                zn = min(512, R - zn0)
                pzc = pstz.tile([CODE, 512], f32, space="PSUM", tag="pzc")
                nc.tensor.matmul(
                    pzc[:, :zn],
                    lhsT=w2s[:, :],
                    rhs=zpreT[:, zn0 : zn0 + zn],
                    start=True,
                    stop=True,
                )
                nc.vector.tensor_copy(zts[:, zn0 : zn0 + zn], pzc[:, :zn])
                zn0 += zn
            nc.sync.dma_start(zt_own[:, :], zts[:, :])

        # own-z strips don't need the collective — prefetch before the AG
        for s in range(4):
            nc.scalar.dma_start(zts4[32 * s : 32 * s + CODE, :], zt_own[:, :])
        nc.gpsimd.collective_compute(
            "AllGather",
            mybir.AluOpType.bypass,
            replica_groups=groups_all,
            ins=[zt_own.opt()],
            outs=[zt_all.opt()],
        )
        for s in range(4):
            nc.sync.dma_start(
                ztall4[32 * s : 32 * s + CODE, :].rearrange(
                    "p (r j) -> p r j", r=NCORES
                ),
                zt_all.rearrange("r p j -> p r j"),
            )

        # ============ decode ============
        with tc.tile_pool(name="obuf", bufs=3) as obuf, tc.tile_pool(
            name="psd", bufs=2, space="PSUM"
        ) as psd:
            qq = 0
            for m in range(MT):
                rm = rows_of(m)
                ob = obuf.tile([128, N], bf16)
                for bg in bank_groups:
                    w = sum(nn for _, nn in bg)
                    pd = psd.tile([128, 2048], f32, space="PSUM")
                    for q, (nn0, nn) in enumerate(bg):
                        s = qq % 4
                        qq += 1
                        p0 = 32 * s
                        nc.tensor.matmul(
                            pd[:rm, q * 512 : q * 512 + nn],
                            lhsT=zts4[p0 : p0 + CODE, m * 128 : m * 128 + rm],
                            rhs=ztall4[p0 : p0 + CODE, nn0 : nn0 + nn],
                            start=True,
                            stop=True,
                            tile_position=(p0, 0),
                        )
                    b0 = bg[0][0]
                    nc.scalar.activation(
                        ob[:rm, b0 : b0 + w], pd[:rm, :w], AF.Sigmoid
                    )
                # rotate output stores over 3 queues (SP/Act HWDGE + SWDGE)
                eng = (nc.sync, nc.scalar, nc.gpsimd)[m % 3]
                eng.dma_start(out_d[m * 128 : m * 128 + rm, :], ob[:rm, :])

    nc.compile()
    return nc


def kernel(x, W1, W2, edge_weight, src, dst, trace=False):
    cfg = Cfg()
    edge_maps = prep(cfg, src, dst, edge_weight)
    x = np.ascontiguousarray(np.asarray(x, dtype=np.float32))
    W1 = np.ascontiguousarray(np.asarray(W1, dtype=np.float32))
    W2 = np.ascontiguousarray(np.asarray(W2, dtype=np.float32))
    ident = np.eye(128, dtype=np.float32)
    in_maps = []
    for c in range(NCORES):
        m = dict(edge_maps[c])
        m.update(
            xs=np.ascontiguousarray(x[c * R : (c + 1) * R]),
            w1=W1,
            w2=W2,
            ident=ident,
        )
        in_maps.append(m)
    nc = build_nc(cfg)
    res = run_bass_kernel_spmd(
        nc, in_maps, core_ids=list(range(NCORES)), trace=trace
    )
    out = np.concatenate(
        [np.asarray(r["out"]).astype(np.float32) for r in res.results], axis=0
    )
    if trace:
        kernel.last_results = res
    return np.ascontiguousarray(out)
